# revision 11
# baseline (speedup 1.0000x reference)
"""Trainium2 Bass kernel for cross-attention.

Reference computation (per batch b):
    q = x @ Wq + bq              # [Lq, D]
    k = cond @ Wk + bk           # [Lk, D]
    v = cond @ Wv + bv           # [Lk, D]
    out = softmax(q @ k.T) @ v   # [Lq, D]   (unscaled dot product)

Shapes: B=4, Lq=Lk=4096, IN_DIM=COND_DIM=256, OUT_DIM=128, fp32.

Sharding: 8 cores; core i owns batch b=i//2 and query rows
[h*2048, (h+1)*2048) with h=i%2, with the full K/V of its batch
(sequence-parallel over Lq, flash-style).

Per-core layout strategy (everything feature-on-partitions):
    xT   [256, 2048]   (PE-transposed x slab)
    condT[256, 4096]
    qT   [128, 2048] = Wq.T @ xT + bq       (ACT adds per-partition bias)
    kT   [128, 4096] = Wk.T @ condT + bk
    vT   [128, 4096] = Wv.T @ condT + bv -> PE-transpose -> v [4096, 128]
    scoresT[s, r] = kT_tile.T @ qT          (s on partitions!)
    expT = exp(scoresT)                     (ScalarE, PSUM->SBUF)
    outT[d, r]  += v_tile.T @ expT          (accumulate over s tiles)
    sums[1, r]  += ones.T @ expT            (softmax denominator via matmul)
    out[r, d] = transpose(outT) * (1/sums)  (per-partition scale on ACT)

Matmuls use dtype float32r (full-rate fp32 on the PE when the moving
free dim is >= 256). All DMA goes through a single SWDGE queue and
compute uses only PE/ACT/DVE: the kernel-tail drain can carry at most
4 semaphore waits (ISA limit), which is exactly PE+ACT+DVE+DMASW0.
"""

import sys
from contextlib import ExitStack

import numpy as np

sys.path.insert(0, "/opt/trn_rl_repo")

import concourse.bass as bass  # noqa: E402
import concourse.tile as tile  # noqa: E402
from concourse import mybir  # noqa: E402
from concourse.bass_utils import run_bass_kernel_spmd  # noqa: E402

B, LQ, LK = 4, 4096, 4096
IN_DIM, COND_DIM, OUT_DIM = 256, 256, 128
P = 128
N_CORES = 8
LQ_SH = LQ * B // N_CORES  # 2048 query rows per core
RC = 512                   # query-chunk (moving free dim of the big matmuls)
N_RC = LQ_SH // RC         # 4
N_S = LK // P              # 32 key tiles
N_CT = COND_DIM // P       # 2 contraction tiles for the projections

FP32 = mybir.dt.float32
FP32R = mybir.dt.float32r
AF = mybir.ActivationFunctionType


def _r(ap):
    """View an fp32 AP as float32r for full-rate PE matmuls."""
    return ap.bitcast(FP32R)



NOP_CHUNK = 1


def _wait_budget(inst):
    # Walrus sync-wait slot budgets differ per lowered ISA struct (S3_LW,
    # S3D3_AC, ... reject multi-wait instructions). Keep one wait on real
    # compute/DMA instructions and hoist the rest onto same-engine NOPs;
    # the CTRL structs behind NoOp/Drain accept several.
    return 1


def _split_excess_waits(nc):
    """Hoist excess semaphore waits onto injected NOPs that precede the
    instruction in the same engine stream — semantically identical, since
    the engine blocks on each wait in order."""
    fn = nc.m.functions[0]
    for bb in fn.blocks:
        new_insts = []
        for inst in bb.instructions:
            si = inst.sync_info
            waits = list(si.on_wait) if si and si.on_wait else []
            budget = _wait_budget(inst)
            if len(waits) > budget:
                extra = waits[:-budget]
                keep = waits[-budget:]
                for i in range(0, len(extra), NOP_CHUNK):
                    chunk = extra[i : i + NOP_CHUNK]
                    nop = mybir.InstNoOp(
                        name=f"{inst.name}-waitsplit{i}",
                        engine=inst.engine,
                        ins=[],
                        outs=[],
                        sync_info=mybir.SyncInfo(on_wait=chunk, on_update=[]),
                    )
                    new_insts.append(nop)
                inst.sync_info = mybir.SyncInfo(
                    on_wait=keep, on_update=list(si.on_update) if si.on_update else []
                )
            new_insts.append(inst)
        bb.instructions[:] = new_insts


def build_program():
    nc = bass.Bass(
        "TRN2", target_bir_lowering=False, debug=False, num_swdge_queues=1
    )
    dt = FP32
    x_d = nc.dram_tensor("x", [LQ_SH, IN_DIM], dt, kind="ExternalInput").ap()
    cond_d = nc.dram_tensor("cond", [LK, COND_DIM], dt, kind="ExternalInput").ap()
    wq_d = nc.dram_tensor("wq", [IN_DIM, OUT_DIM], dt, kind="ExternalInput").ap()
    wk_d = nc.dram_tensor("wk", [COND_DIM, OUT_DIM], dt, kind="ExternalInput").ap()
    wv_d = nc.dram_tensor("wv", [COND_DIM, OUT_DIM], dt, kind="ExternalInput").ap()
    bq_d = nc.dram_tensor("bq", [OUT_DIM], dt, kind="ExternalInput").ap()
    bk_d = nc.dram_tensor("bk", [OUT_DIM], dt, kind="ExternalInput").ap()
    bv_d = nc.dram_tensor("bv", [OUT_DIM], dt, kind="ExternalInput").ap()
    ident_d = nc.dram_tensor("ident", [P, P], dt, kind="ExternalInput").ap()
    ones_d = nc.dram_tensor("ones", [P, 1], dt, kind="ExternalInput").ap()
    out_d = nc.dram_tensor("out", [LQ_SH, OUT_DIM], dt, kind="ExternalOutput").ap()

    with tile.TileContext(nc) as tc, ExitStack() as ctx:
        dma = nc.gpsimd.dma_start  # single SWDGE queue

        consts = ctx.enter_context(tc.tile_pool(name="consts", bufs=1))
        acts = ctx.enter_context(tc.tile_pool(name="acts", bufs=1))

        ident = consts.tile([P, P], dt)
        dma(out=ident, in_=ident_d)
        ones = consts.tile([P, 1], dt)
        dma(out=ones, in_=ones_d)
        w_sb = {}
        for name, w_d in (("wq", wq_d), ("wk", wk_d), ("wv", wv_d)):
            for j in range(N_CT):
                raw = consts.tile([P, OUT_DIM], dt, name=f"{name}{j}raw")
                dma(out=raw, in_=w_d[j * P : (j + 1) * P, :])
                t = consts.tile([P, OUT_DIM], dt, name=f"{name}{j}")
                nc.vector.tensor_copy(_r(t), raw)
                w_sb[name, j] = t
        ones_r = consts.tile([P, 1], dt)
        b_sb = {}
        for name, bias_d in (("bq", bq_d), ("bk", bk_d), ("bv", bv_d)):
            t = consts.tile([P, 1], dt, name=name)
            dma(out=t, in_=bias_d.unsqueeze(1))
            b_sb[name] = t

        # Load the exp table set before anything else runs on ACT so the
        # mid-kernel PSEUDO_LOAD_ACT_FUNC_SET stall lands at t=0.
        warm = consts.tile([P, 1], dt)
        nc.scalar.activation(warm, ones, AF.Exp)
        nc.vector.tensor_copy(_r(ones_r), ones)

        # Persistent activations.
        qT = acts.tile([P, LQ_SH], dt)
        kT = acts.tile([P, LK], dt)
        v_sb = acts.tile([P, N_S * P], dt)  # v tile s at [:, s*128:(s+1)*128]

        # ---------------- Prologue: transposes + projections ----------------
        with (
            tc.tile_pool(name="stage", bufs=1) as stage,
            tc.tile_pool(name="pro_T", bufs=1) as pro_T,
            tc.tile_pool(name="ps_tr", bufs=3, space="PSUM") as ps_tr,
            tc.tile_pool(name="ps_mm", bufs=3, space="PSUM") as ps_mm,
        ):
            # Stage raw x/cond: [128, n_blocks, C] with row-block index in
            # the free dim; one big DMA each.
            x_st = stage.tile([P, LQ_SH // P, IN_DIM], dt)
            dma(out=x_st, in_=x_d.rearrange("(i p) c -> p i c", p=P))
            c_st = stage.tile([P, LK // P, COND_DIM], dt)
            dma(out=c_st, in_=cond_d.rearrange("(i p) c -> p i c", p=P))

            xT = [pro_T.tile([P, LQ_SH], dt, name=f"xT{j}") for j in range(N_CT)]
            condT = [pro_T.tile([P, LK], dt, name=f"condT{j}") for j in range(N_CT)]

            # PE transposes, flushed to SBUF four 128x128 blocks at a time.
            for dst, src, nblk in ((xT, x_st, LQ_SH // P), (condT, c_st, LK // P)):
                for j in range(N_CT):
                    for g in range(nblk // 4):
                        tp = ps_tr.tile([P, 4 * P], dt, name="tp")
                        for u in range(4):
                            i = g * 4 + u
                            nc.tensor.transpose(
                                tp[:, u * P : (u + 1) * P],
                                src[:, i, j * P : (j + 1) * P],
                                ident,
                            )
                        nc.vector.tensor_copy(
                            _r(dst[j][:, g * 4 * P : (g + 1) * 4 * P]), tp
                        )

            # qT / kT / vT projections (+bias fused into the PSUM->SBUF copy).
            vT = pro_T.tile([P, LK], dt)
            for dst, w, bias, src, length in (
                (qT, "wq", "bq", xT, LQ_SH),
                (kT, "wk", "bk", condT, LK),
                (vT, "wv", "bv", condT, LK),
            ):
                for rc in range(length // RC):
                    pq = ps_mm.tile([P, RC], dt, name="pq")
                    for j in range(N_CT):
                        nc.tensor.matmul(
                            pq,
                            _r(w_sb[w, j]),
                            _r(src[j][:, rc * RC : (rc + 1) * RC]),
                            start=(j == 0),
                            stop=(j == N_CT - 1),
                        )
                    nc.scalar.activation(
                        _r(dst[:, rc * RC : (rc + 1) * RC]),
                        pq,
                        AF.Identity,
                        bias=b_sb[bias],
                    )

            # v natural [s, d] tiles from vT via PE transpose.
            for g in range(N_S // 4):
                tp = ps_tr.tile([P, 4 * P], dt, name="tp")
                for u in range(4):
                    s = g * 4 + u
                    nc.tensor.transpose(
                        tp[:, u * P : (u + 1) * P],
                        vT[:, s * P : (s + 1) * P],
                        ident,
                    )
                nc.vector.tensor_copy(_r(v_sb[:, g * 4 * P : (g + 1) * 4 * P]), tp)

        # ---------------- Main attention loop ----------------
        with (
            tc.tile_pool(name="ps_sc", bufs=2, space="PSUM") as ps_sc,
            tc.tile_pool(name="ps_out", bufs=2, space="PSUM") as ps_out,
            tc.tile_pool(name="ps_sum", bufs=2, space="PSUM") as ps_sum,
            tc.tile_pool(name="ps_epi", bufs=1, space="PSUM") as ps_epi,
            tc.tile_pool(name="expp", bufs=4) as expp,
            tc.tile_pool(name="episb", bufs=2) as episb,
        ):
            for rc in range(N_RC):
                q_mv = _r(qT[:, rc * RC : (rc + 1) * RC])
                out_ps = ps_out.tile([P, RC], dt, name="out_ps")
                sum_ps = ps_sum.tile([1, RC], dt, name="sum_ps")
                for s in range(N_S):
                    sc_ps = ps_sc.tile([P, RC], dt, name="sc_ps")
                    nc.tensor.matmul(
                        sc_ps, _r(kT[:, s * P : (s + 1) * P]), q_mv
                    )
                    expT = expp.tile([P, RC], dt, name="expT")
                    nc.scalar.activation(_r(expT), sc_ps, AF.Exp)
                    nc.tensor.matmul(
                        out_ps,
                        _r(v_sb[:, s * P : (s + 1) * P]),
                        _r(expT),
                        start=(s == 0),
                        stop=(s == N_S - 1),
                    )
                    nc.tensor.matmul(
                        sum_ps,
                        _r(ones_r),
                        _r(expT),
                        start=(s == 0),
                        stop=(s == N_S - 1),
                    )

                # Epilogue: normalize + transpose back to [r, d].
                recip = episb.tile([1, RC], dt, name="recip")
                nc.vector.reciprocal(recip, sum_ps)
                rT_ps = ps_epi.tile([P, RC // P], dt, name="rT_ps")
                for j in range(RC // P):
                    nc.tensor.transpose(
                        rT_ps[:, j : j + 1],
                        recip[:, j * P : (j + 1) * P],
                        ident[0:1, 0:1],
                    )
                recipT = episb.tile([P, RC // P], dt, name="recipT")
                nc.vector.tensor_copy(recipT, rT_ps)

                outT_sb = episb.tile([P, RC], dt, name="outT_sb")
                nc.scalar.copy(outT_sb, out_ps)
                tr_ps = ps_epi.tile([P, RC], dt, name="tr_ps")
                for j in range(RC // P):
                    nc.tensor.transpose(
                        tr_ps[:, j * P : (j + 1) * P],
                        outT_sb[:, j * P : (j + 1) * P],
                        ident,
                    )
                outf = episb.tile([P, RC], dt, name="outf")
                for j in range(RC // P):
                    nc.scalar.mul(
                        outf[:, j * P : (j + 1) * P],
                        tr_ps[:, j * P : (j + 1) * P],
                        recipT[:, j : j + 1],
                    )
                dma(
                    out=out_d[rc * RC : (rc + 1) * RC, :].rearrange(
                        "(j p) d -> p j d", p=P
                    ),
                    in_=outf.rearrange("p (j d) -> p j d", d=OUT_DIM),
                )
    return nc


_NC = None


def _get_program():
    global _NC
    if _NC is None:
        _NC = build_program()
        _split_excess_waits(_NC)
    return _NC


def make_in_maps(x, cond, Wq, bq, Wk, bk, Wv, bv):
    aux = {
        "wq": np.ascontiguousarray(Wq, np.float32),
        "wk": np.ascontiguousarray(Wk, np.float32),
        "wv": np.ascontiguousarray(Wv, np.float32),
        "bq": np.ascontiguousarray(bq, np.float32),
        "bk": np.ascontiguousarray(bk, np.float32),
        "bv": np.ascontiguousarray(bv, np.float32),
        "ident": np.eye(P, dtype=np.float32),
        "ones": np.ones((P, 1), np.float32),
    }
    in_maps = []
    for core in range(N_CORES):
        b, h = divmod(core, 2)
        in_maps.append(
            {
                "x": np.ascontiguousarray(
                    x[b, h * LQ_SH : (h + 1) * LQ_SH, :], np.float32
                ),
                "cond": np.ascontiguousarray(cond[b], np.float32),
                **aux,
            }
        )
    return in_maps


def kernel(x, cond, Wq, bq, Wk, bk, Wv, bv):
    x = np.asarray(x, np.float32)
    cond = np.asarray(cond, np.float32)
    nc = _get_program()
    in_maps = make_in_maps(x, cond, Wq, bq, Wk, bk, Wv, bv)
    res = run_bass_kernel_spmd(nc, in_maps, list(range(N_CORES)))
    kernel._last_results = res
    out = np.empty((B, LQ, OUT_DIM), np.float32)
    for core in range(N_CORES):
        b, h = divmod(core, 2)
        out[b, h * LQ_SH : (h + 1) * LQ_SH, :] = res.results[core]["out"]
    return out
